# revision 6
# baseline (speedup 1.0000x reference)
"""Trainium2 Bass kernel for nn_CustomGINConv (gnn_message_passing).

Reference computation (per path n, L=6 layers, C=128 channels):
    h[l]    = x[l] @ Wt[:C] + emb[idx[l]] @ Wt[C:] + bt
    prop[l] = h[l-1] + h[l+1]                (zero-padded along l)
    u[l]    = (1+eps) * x[l] + prop[l]
    out     = sum_l relu(u[l] @ W1 + b1) @ W2 + b2   -> [N, C]

Strategy (shard N across 8 cores, feature-major on-chip layout):
  * Everything linear before the relu is folded host-side. With
    T = emb @ Wt[C:] + bt and s = 1+eps (the eps scales cancel):
      z1[l] = x[l] @ (s*W1) + (x[l-1]+x[l+1]) @ (Wt[:C] @ W1)
              + ohsum[l] @ (T @ W1) + b1
  * The embedding gather is a one-hot matmul in fp8 (values {0,1,2}/16,
    exact) against a 16x-scaled (T @ W1) table. b1 is folded into that
    table: a 101st dummy row gives every column exactly two hots, and
    each table row carries +8*b1, so 2 x (1/16) x 8*b1 = b1 accumulates
    in PSUM for free.
  * W2 linearity: out = (sum_l relu(z1[l])) @ W2 + L*b2. Relu outputs
    are summed in PAIRS before W2 (3 W2 matmuls per tile instead of 6):
    s_k = relu(z[2k]) + relu(z[2k+1]) as one ACT relu plus one DVE
    scalar_tensor_tensor (fused max(z,0)+add). PE per tile is 21 dense
    matmuls (down from 24); the TimelineSim schedule has zero PE gaps
    (71.6us/pass steady, HW med-delta measures ~70us vs baseline ~81).
  * Cross-tile software pipelining: tile i's W2 matmuls are emitted
    mid-way through tile i+1's z-matmuls so their pair-sum inputs are a
    full tile old and PE never waits on the relu/add chain.
  * x path and output are float16 (same speed/bytes as bf16, 4 more
    mantissa bits -> rel err ~9e-4). Output DMA is halved vs f32.
  * DoubleRow fp8 perf mode is OFF (KERNEL_DR=1 to enable): its
    LDWEIGHTS pattern destabilizes the scheduler and measures slower
    despite fewer PE cycles on paper. KERNEL_PRESUM=1 (single W2 matmul
    per tile) fails neuronxcc codegen today - do not enable.
"""

import os
import sys

import numpy as np

sys.path.insert(0, "/opt/trn_rl_repo")

import ml_dtypes  # noqa: E402

import concourse.bass as bass  # noqa: E402
import concourse.tile as tile  # noqa: E402
from concourse import bacc, mybir  # noqa: E402
from concourse import bass_utils  # noqa: E402
from concourse.bass import ts  # noqa: E402

L = 6
N_FULL = 65536
C = 128
EMB = 100
EMB2 = 102  # +1 dummy-hot row for the b1 fold, +1 pad to even for DoubleRow
EMB2_H = EMB2 // 2
NCORES = 8
NC_N = N_FULL // NCORES  # 8192 rows per core
M = 512  # tile width (columns of the feature-major layout)

F32 = mybir.dt.float32
F32R = mybir.dt.float32r
BF16 = mybir.dt.bfloat16
F16 = mybir.dt.float16
F8 = mybir.dt.float8e4

RELU = mybir.ActivationFunctionType.Relu
IDENT = mybir.ActivationFunctionType.Identity

YDEMOTE = int(os.environ.get("KERNEL_YDEMOTE", "0"))
USE_DR = os.environ.get("KERNEL_DR", "0") == "1"
XS_POOL = os.environ.get("KERNEL_XS_POOL", "0") == "1"
PRESUM = os.environ.get("KERNEL_PRESUM", "0") == "1"

# fp8e4m3 bit patterns for {0, 1/16, 2/16}: hots carry a 1/16 factor
# (exact powers of two) and tw1 is pre-scaled by 16 so its small entries
# sit in fp8's normal range instead of the subnormals.
_FP8_LUT = np.array([0x00, 0x18, 0x20], dtype=np.uint8)
_OH_SCALE = np.float32(16.0)


def build_bass(nc_n: int = NC_N, num_devices: int = NCORES,
               repeat: int = 1, hw_loop: bool = False) -> bass.Bass:
    """Build + compile the per-core Bass program (same program on all cores).

    repeat>1 re-runs the whole tile loop (for timing: on-device work scales
    by `repeat` while dispatch overhead stays fixed).  With hw_loop=True the
    repetition is a tc.For_i hardware loop (tiny NEFF, huge repeat counts
    for repeat-delta timing); the per-pass cost then includes the Tile
    back-edge barrier, approximating a full single-NEFF pass."""
    nc = bacc.Bacc(
        "TRN2",
        target_bir_lowering=False,
        debug=False,
        enable_asserts=False,
        num_devices=num_devices,
    )
    oh_shape = [EMB2_H, 2, L, nc_n] if USE_DR else [EMB2, L, nc_n]
    tw1_shape = [EMB2_H, 2, C] if USE_DR else [EMB2, C]
    xt = nc.dram_tensor("xt", [C, L, nc_n], F16, kind="ExternalInput").ap()
    oh = nc.dram_tensor("oh", oh_shape, F8, kind="ExternalInput").ap()
    w1d = nc.dram_tensor("w1d", [C, C], F16, kind="ExternalInput").ap()
    w1x = nc.dram_tensor("w1x", [C, C], F16, kind="ExternalInput").ap()
    tw1 = nc.dram_tensor("tw1", tw1_shape, F8, kind="ExternalInput").ap()
    w2 = nc.dram_tensor("w2", [C, C], F32R, kind="ExternalInput").ap()
    b2s = nc.dram_tensor("b2s", [C, 1], F32, kind="ExternalInput").ap()
    out = nc.dram_tensor("out", [C, nc_n], F16, kind="ExternalOutput").ap()

    nt = nc_n // M
    with tile.TileContext(nc) as tc:
        with (
            tc.tile_pool(name="consts", bufs=1) as consts,
            tc.tile_pool(name="xp", bufs=3) as xp,
            tc.tile_pool(name="ohp", bufs=3) as ohp,
            tc.tile_pool(name="zp", bufs=3) as zp,
            tc.tile_pool(name="outp", bufs=2) as outp,
            tc.tile_pool(name="pp", bufs=1, space="PSUM") as pp,
        ):
            w1d_sb = consts.tile([C, C], F16, tag="w1d")
            nc.scalar.dma_start(w1d_sb[:], w1d)
            w1x_sb = consts.tile([C, C], F16, tag="w1x")
            nc.scalar.dma_start(w1x_sb[:], w1x)
            tw1_sb = consts.tile(tw1_shape, F8, tag="tw1")
            nc.scalar.dma_start(tw1_sb[:], tw1)
            w2_sb = consts.tile([C, C], F32R, tag="w2")
            nc.scalar.dma_start(w2_sb[:], w2)
            b2_sb = consts.tile([C, 1], F32, tag="b2")
            nc.scalar.dma_start(b2_sb[:], b2s)

            def one_pass(first_pass: bool):
              pending = None
              for i_rep in range(nt):
                i = i_rep % nt
                xt_t = xp.tile([C, L, M], F16, tag="xt")
                oh_t = ohp.tile(
                    [EMB2_H, 2, L, M] if USE_DR else [EMB2, L, M], F8, tag="oh"
                )

                def oh_slice(t, l):
                    return t[:, :, l, :] if USE_DR else t[:, l, :]

                if i_rep == 0 and first_pass:
                    # split the very first loads per layer so l=0's matmuls
                    # start as soon as x[0], x[1], ohsum[0] land instead of
                    # waiting for the full tile
                    for l in range(L):
                        nc.sync.dma_start(xt_t[:, l, :], xt[:, l, ts(i, M)])
                        nc.sync.dma_start(oh_slice(oh_t, l), oh_slice(oh, l)[..., ts(i, M)])
                else:
                    nc.sync.dma_start(xt_t[:], xt[..., ts(i, M)])
                    nc.sync.dma_start(oh_t[:], oh[..., ts(i, M)])

                # xs[l] = x[l-1] + x[l+1] for interior l (one stacked DVE op);
                # boundary layers use the single neighbor directly.
                xs_t = xp.tile([C, L - 2, M], F16, tag="xs")
                xs_eng = nc.gpsimd if XS_POOL else nc.vector
                xs_eng.tensor_tensor(
                    xs_t[:], xt_t[:, 0 : L - 2, :], xt_t[:, 2:L, :],
                    mybir.AluOpType.add,
                )

                def z_matmuls(z_ps, l):
                    # z1[l] (pre-relu, incl b1 via the oh dummy-hot fold).
                    # DR matmul opens the accumulation group so the bank can
                    # close early even after the scheduler groups by weight.
                    nc.tensor.matmul(
                        z_ps[:], tw1_sb[:], oh_slice(oh_t, l),
                        start=True, stop=False,
                        perf_mode=(
                            mybir.MatmulPerfMode.DoubleRow if USE_DR else None
                        ),
                    )
                    nc.tensor.matmul(
                        z_ps[:], w1d_sb[:], xt_t[:, l, :], start=False, stop=False
                    )
                    nbr = (
                        xt_t[:, 1, :] if l == 0
                        else xt_t[:, L - 2, :] if l == L - 1
                        else xs_t[:, l - 1, :]
                    )
                    nc.tensor.matmul(z_ps[:], w1x_sb[:], nbr, start=False, stop=True)

                # cross-tile software pipelining: tile i's z-matmuls run on
                # PE while ACT/DVE turn them into pair sums s_k; the W2
                # (y) matmuls for tile i-1 are issued after tile i's
                # z-matmuls, so their s_k inputs are long ready and PE never
                # stalls on the relu/add chain.
                s_sbs = []
                for k in range(3):
                    if k == 2 and pending is not None:
                        # slot the previous tile's W2 matmuls mid-stream:
                        # their s inputs are a full tile old, so PE takes
                        # them with zero wait, and this tile's tail flows
                        # straight into the next tile's z-matmuls.
                        emit_y(*pending)
                        pending = None
                    la, lb = 2 * k, 2 * k + 1
                    za = pp.tile([C, M], F32, tag="z", bufs=6)
                    z_matmuls(za, la)
                    ra = zp.tile([C, M], F32, tag="ra", bufs=3)
                    nc.scalar.activation(ra[:], za[:], RELU)
                    zb = pp.tile([C, M], F32, tag="z", bufs=6)
                    z_matmuls(zb, lb)
                    s_sb = zp.tile([C, M], F16 if PRESUM else F32R, tag="s", bufs=6)
                    # s = max(zb, 0) + ra   (fused relu+add on DVE)
                    nc.vector.scalar_tensor_tensor(
                        s_sb[:], zb[:], 0.0, ra[:],
                        mybir.AluOpType.max, mybir.AluOpType.add,
                    )
                    s_sbs.append(s_sb)

                def emit_y(prev_i, prev_s):  # noqa: B023
                    # demote priority so the scheduler keeps the W2 matmuls
                    # interleaved with the NEXT tile's z-matmuls instead of
                    # hoisting them back to their own tile's tail
                    p0 = tc.cur_priority
                    tc.cur_priority = p0 + YDEMOTE
                    y_ps = pp.tile([C, M], F32, tag="y", bufs=2)
                    out_t = outp.tile([C, M], F16, tag="out")
                    for k, s_sb in enumerate(prev_s):
                        nc.tensor.matmul(
                            y_ps[:], w2_sb[:], s_sb[:],
                            start=(k == 0), stop=(k == len(prev_s) - 1),
                        )
                    nc.scalar.activation(out_t[:], y_ps[:], IDENT, bias=b2_sb[:])
                    nc.sync.dma_start(out[:, ts(prev_i, M)], out_t[:])
                    tc.cur_priority = max(p0 + 6, tc.cur_priority - YDEMOTE)

                if PRESUM:
                    m0 = zp.tile([C, M], F16, tag="m0", bufs=2)
                    nc.vector.tensor_tensor(
                        m0[:], s_sbs[0][:], s_sbs[1][:], mybir.AluOpType.add
                    )
                    racc = zp.tile([C, M], F16, tag="racc", bufs=3)
                    nc.vector.tensor_tensor(
                        racc[:], m0[:], s_sbs[2][:], mybir.AluOpType.add
                    )
                    s_sbs = [racc]
                pending = (i, s_sbs)
              if pending is not None:
                emit_y(*pending)

            if hw_loop and repeat > 1:
                with tc.For_i(0, repeat, 1):
                    one_pass(first_pass=False)
            else:
                for r in range(repeat):
                    one_pass(first_pass=(r == 0))

    nc.compile()
    return nc


def prep_host(x, atomic_type, emb, Wt, bt, eps, W1, b1, W2, b2, nc_n=NC_N,
              ncores=NCORES):
    """Host-side prep: fold eps/b1 into weights, build per-core input maps."""
    x = np.asarray(x, dtype=np.float32)
    idx = np.asarray(atomic_type).astype(np.int64)
    emb = np.asarray(emb, dtype=np.float32)
    Wt = np.asarray(Wt, dtype=np.float32)
    bt = np.asarray(bt, dtype=np.float32)
    W1 = np.asarray(W1, dtype=np.float32)
    b1 = np.asarray(b1, dtype=np.float32)
    W2 = np.asarray(W2, dtype=np.float32)
    b2 = np.asarray(b2, dtype=np.float32)
    scale = 1.0 + np.float32(np.asarray(eps).reshape(-1)[0])

    # W1 folded through the propagate step (eps-scales cancel in the products):
    #   z1[l] = x[l] @ (scale*W1) + x[l+/-1] @ (Wt[:C] @ W1) + ohsum[l] @ (T @ W1)
    # with T = emb @ Wt[C:] + bt.
    T = (emb @ Wt[C:]) + bt  # [EMB, C]
    w1d = np.ascontiguousarray((W1 * scale).astype(np.float16))
    w1x = np.ascontiguousarray(
        (Wt[:C].astype(np.float64) @ W1.astype(np.float64)).astype(np.float16)
    )
    # table rows: 16*(T@W1) for e<100, row 100 = dummy (0), row 101 = pad;
    # every row offset by +8*b1 so two 1/16-hots per column contribute b1.
    tw1f = np.zeros((EMB2, C), dtype=np.float64)
    tw1f[:EMB] = _OH_SCALE * (T.astype(np.float64) @ W1.astype(np.float64))
    tw1f += 8.0 * b1.astype(np.float64)
    tw1 = tw1f.astype(ml_dtypes.float8_e4m3)
    if USE_DR:
        tw1 = tw1.reshape(EMB2_H, 2, C)
    tw1 = np.ascontiguousarray(tw1)
    w2s = np.ascontiguousarray(W2)
    b2s = np.ascontiguousarray((np.float32(L) * b2).reshape(C, 1))

    arange_emb = np.arange(EMB, dtype=idx.dtype)
    in_maps = []
    for k in range(ncores):
        n0 = k * nc_n
        xs = x[:, n0 : n0 + nc_n, :]  # [L, nc_n, C]
        xtk = np.ascontiguousarray(xs.transpose(2, 0, 1)).astype(
            np.float16
        )  # [C, L, nc_n]
        ii = idx[:, n0 : n0 + nc_n]  # [L, nc_n]
        ohb = (ii[:, None, :] == arange_emb[None, :, None]).view(np.uint8)
        ohs = np.zeros((L, EMB2, nc_n), dtype=np.uint8)
        ohs[:-1, :EMB] += ohb[1:]
        ohs[1:, :EMB] += ohb[:-1]
        ohs[0, EMB] += 1   # dummy hot so boundary columns also carry 2 hots
        ohs[L - 1, EMB] += 1
        ohk = _FP8_LUT[ohs.transpose(1, 0, 2)]  # [EMB2, L, nc_n] uint8 bits
        ohk = np.ascontiguousarray(ohk).view(ml_dtypes.float8_e4m3)
        if USE_DR:
            ohk = ohk.reshape(EMB2_H, 2, L, nc_n)
        in_maps.append(
            {
                "xt": xtk,
                "oh": ohk,
                "w1d": w1d,
                "w1x": w1x,
                "tw1": tw1,
                "w2": w2s,
                "b2s": b2s,
            }
        )
    return in_maps


_COMPILED = {}


def get_compiled(nc_n=NC_N, num_devices=NCORES):
    key = (nc_n, num_devices)
    if key not in _COMPILED:
        _COMPILED[key] = build_bass(nc_n, num_devices)
    return _COMPILED[key]


def run_on_hw(in_maps, nc=None, trace=False, **kwargs):
    if nc is None:
        nc = get_compiled()
    return bass_utils.run_bass_kernel_spmd(
        nc, in_maps, core_ids=list(range(len(in_maps))), trace=trace, **kwargs
    )


def kernel(**inputs) -> np.ndarray:
    in_maps = prep_host(
        inputs["x"],
        inputs["atomic_type"],
        inputs["emb"],
        inputs["Wt"],
        inputs["bt"],
        inputs["eps"],
        inputs["W1"],
        inputs["b1"],
        inputs["W2"],
        inputs["b2"],
    )
    res = run_on_hw(in_maps)
    out = np.empty((N_FULL, C), dtype=np.float32)
    for k in range(NCORES):
        out[k * NC_N : (k + 1) * NC_N, :] = (
            res.results[k]["out"].astype(np.float32).T
        )
    return out


if __name__ == "__main__":
    import reference  # only when run manually inside /root/problem

    inputs = {k: np.asarray(v) for k, v in reference.setup_inputs().items()}
    got = kernel(**inputs)
    want = np.asarray(reference.reference(**inputs))
    err = np.abs(got - want).max() / np.abs(want).max()
    print("rel err:", err)

